# revision 1
# baseline (speedup 1.0000x reference)
"""Causal self-attention (B=2, T=2048, C=1024, H=16) on 8 TRN2 NeuronCores.

Sharding: core = b*4 + g  (b in 0..1 batches, g in 0..3 head-groups of 4 heads).
Each core computes QKV for its 4 heads (tensor-parallel columns of W_attn),
full causal attention over T=2048, and a partial projection
y_g @ W_proj[rows_g] -> [T, C].  Host sums the 4 partials per batch and adds
b_proj.

Device layout notes:
  - x is pre-transposed on host to xT [C, T] so all matmuls contract over
    partitions.
  - sim is computed transposed: simT[tk, tq] = k . q   (lhsT=kT, rhs=qT),
    exp via ScalarE (scale=1/sqrt(C) folded in), causal mask applied as a
    gpsimd affine_select (exact zeros) on the exp tiles.
  - attn@v: lhsT = [v | ones] per head -> psum rows 0..63 = unnormalized y^T,
    row 64 = softmax denominator.  DVE reciprocal -> tiny SBUF DMA to
    partition 0 -> gpsimd partition_broadcast yields a [64, 512] SBUF tile
    aligned with y^T; one DVE multiply evicts normalized y^T to SBUF bf16.
    (PE-broadcast fallback kept under use_pbcast=False.)
  - y^T is packed two heads per 128 partitions (odd heads placed via a small
    SBUF->SBUF DMA) so the projection contracts K=128.
  - all matmul inputs bf16 (host-cast), accumulation f32, partial outputs
    bf16 (summed in f32 on host).
"""

import sys

sys.path.insert(0, "/opt/trn_rl_repo")

import numpy as np
import ml_dtypes

BF16 = ml_dtypes.bfloat16

B, T, C = 2, 2048, 1024
H, D = 16, 64
HPC = 4          # heads per core
GC = HPC * D     # head-group channel width (256)
NT = T // 128    # 16 row tiles
NS = T // 512    # 4 query super-tiles

_cached = None


def _build(repeat=1, use_pbcast=True):
    # note: tensor_mul with both operands in PSUM is rejected by the walrus
    # BIR verifier, so the non-pbcast path uses a two-step evict.
    import concourse.bass as bass  # noqa: F401
    import concourse.mybir as mybir
    import concourse.tile as tile
    from concourse import bacc

    f32 = mybir.dt.float32
    bf16 = mybir.dt.bfloat16
    AF = mybir.ActivationFunctionType

    nc = bacc.Bacc(None, target_bir_lowering=False, debug=False)
    xt_d = nc.declare_dram_parameter("xt", [C, T], bf16, isOutput=False)
    wqk_d = nc.declare_dram_parameter("wqk", [C, 2 * GC], bf16, isOutput=False)
    wv_d = nc.declare_dram_parameter("wv", [C, GC], bf16, isOutput=False)
    wp_d = nc.declare_dram_parameter("wp", [GC, C], bf16, isOutput=False)
    bqk_d = nc.declare_dram_parameter("bqk", [2 * GC], f32, isOutput=False)
    bv_d = nc.declare_dram_parameter("bv", [GC], f32, isOutput=False)
    out_d = nc.declare_dram_parameter("out", [T, C], bf16, isOutput=True)

    with tile.TileContext(nc) as tc:
        with (
            tc.tile_pool(name="const", bufs=1) as cpool,
            tc.tile_pool(name="exp", bufs=4) as epool,
            tc.tile_pool(name="ostg", bufs=2) as opool,
            tc.tile_pool(name="smal", bufs=3) as spool,
            tc.tile_pool(name="mm", bufs=2, space="PSUM") as mmp,
            tc.tile_pool(name="sim", bufs=2, space="PSUM") as simp_pool,
            tc.tile_pool(name="yp", bufs=2, space="PSUM") as ypp,
        ):
            yta_ref = [None]
            wp_ref = [None]

            def emit_once():
                # ---- inputs: weights first (small), then xT spread over
                # several DMAs ----
                wqk_re = wqk_d[:].rearrange("(c p) m -> p c m", p=128)
                wqka = cpool.tile([128, 4, 2 * GC], bf16, tag="wqka")
                nc.sync.dma_start(out=wqka[:], in_=wqk_re[:, 0:4, :])
                wqkb = cpool.tile([128, 4, 2 * GC], bf16, tag="wqkb")
                nc.sync.dma_start(out=wqkb[:], in_=wqk_re[:, 4:8, :])

                def wqk_slice(c, sl):
                    t_ = wqka if c < 4 else wqkb
                    return t_[:, c % 4, sl]
                # x^T split per query super-tile so s=0 compute starts after
                # ~1MB of DMA instead of the full 4MB; the first chunk is
                # further halved so the very first matmuls start sooner
                xt_parts = []
                xt_re = xt_d[:].rearrange("(c p) t -> p c t", p=128)
                for s in range(NS):
                    if s == 0:
                        xa = cpool.tile([128, 4, 512], bf16, tag="xt0a")
                        nc.sync.dma_start(out=xa[:], in_=xt_re[:, 0:4, 0:512])
                        xb = cpool.tile([128, 4, 512], bf16, tag="xt0b")
                        nc.sync.dma_start(out=xb[:], in_=xt_re[:, 4:8, 0:512])
                        xt_parts.append([(xa, 0), (xb, 4)])
                        bqk = cpool.tile([128, 4], f32, tag="bqk")
                        nc.sync.dma_start(
                            out=bqk[:], in_=bqk_d[:].rearrange("(m p) -> p m", p=128)
                        )
                        wv = cpool.tile([128, 8, GC], bf16, tag="wv")
                        nc.sync.dma_start(
                            out=wv[:],
                            in_=wv_d[:].rearrange("(c p) m -> p c m", p=128),
                        )
                    else:
                        x_s = cpool.tile([128, 8, 512], bf16, tag=f"xt{s}")
                        nc.sync.dma_start(
                            out=x_s[:],
                            in_=xt_re[:, :, s * 512 : (s + 1) * 512],
                        )
                        xt_parts.append([(x_s, 0)])

                def xslice(s, c, sl):
                    for t_, c0 in xt_parts[s]:
                        if c0 <= c < c0 + 4 or (c0 == 0 and len(xt_parts[s]) == 1):
                            return t_[:, c - c0, sl]
                    raise AssertionError
                # W_proj rows for the head pair j live at partitions
                # [0..127] = channels j*128..j*128+127
                wp = cpool.tile([128, 2, C], bf16, tag="wp")
                wp_ref[0] = wp
                nc.sync.dma_start(
                    out=wp[:], in_=wp_d[:].rearrange("(j p) n -> p j n", p=128)
                )
                bv1 = cpool.tile([1, GC], f32, tag="bv1")
                nc.sync.dma_start(
                    out=bv1[:], in_=bv_d[:].rearrange("(o v) -> o v", o=1)
                )

                ones = cpool.tile([1, 128], f32, tag="ones")
                nc.any.memset(ones[:], 1.0)
                # ones row at partition 64 for the denominator broadcast
                ones64 = cpool.tile([65, 64], f32, tag="ones64")
                nc.any.memset(ones64[64:65, :], 1.0)
                zbias = cpool.tile([128, 1], f32, tag="zbias")
                nc.any.memset(zbias[:], 0.0)

                # ---- QKV tiles + attention, interleaved per query super-tile
                # so ScalarE exp work starts as early as possible ----
                qkT = cpool.tile([128, 4, T], bf16, tag="qkT")
                bvb = cpool.tile([128, GC], f32, tag="bvb")
                v1 = cpool.tile([128, NT, HPC, 65], bf16, tag="v1")
                nc.gpsimd.memset(v1[:, :, :, 64:65], 1.0)
                # y^T packed 2 heads per 128 partitions: [128, pair, T]
                yta = cpool.tile([128, 2, T], bf16, tag="yta")
                yta_ref[0] = yta

                for s in range(NS):
                    # q^T,k^T columns for this super-tile
                    for m in range(4):
                        ps = mmp.tile([128, 512], f32, tag="mm")
                        for c in range(8):
                            nc.tensor.matmul(
                                ps[:],
                                wqk_slice(c, slice(m * 128, (m + 1) * 128)),
                                xslice(s, c, slice(None)),
                                start=(c == 0),
                                stop=(c == 7),
                            )
                        nc.vector.tensor_scalar_add(
                            qkT[:, m, s * 512 : (s + 1) * 512],
                            ps[:],
                            bqk[:, m : m + 1],
                        )
                    if s == 0:
                        # broadcast b_v across partitions via K=1 matmul
                        pbv = mmp.tile([128, GC], f32, tag="mm")
                        nc.tensor.matmul(
                            pbv[:], ones[:, 0:128], bv1[:], start=True, stop=True
                        )
                        nc.vector.tensor_copy(bvb[:], pbv[:])
                    # v rows for this super-tile (+ ones column)
                    for t in range(s * 4, s * 4 + 4):
                        ps = mmp.tile([128, GC], f32, tag="mm")
                        for c in range(8):
                            nc.tensor.matmul(
                                ps[:],
                                xslice(
                                    s,
                                    c,
                                    slice((t - 4 * s) * 128, (t - 4 * s + 1) * 128),
                                ),
                                wv[:, c, :],
                                start=(c == 0),
                                stop=(c == 7),
                            )
                        nc.vector.tensor_add(
                            v1[:, t, :, 0:64],
                            ps[:].rearrange("p (l d) -> p l d", d=64),
                            bvb[:].rearrange("p (l d) -> p l d", d=64),
                        )

                    for li, l in enumerate((0, 1, 3, 2)):
                        if s > 0 and li in (1, 2):
                            # projection for the previous super-tile, deferred
                            # and split across two heads so its y^T inputs
                            # (incl. the odd-head SBUF DMA) have settled and
                            # the mm-psum pool isn't hammered all at once
                            emit_proj(s - 1, (0, 1) if li == 1 else (2, 3))
                        poff = (l % 2) * 64
                        qt = l // 2
                        q_ap = qkT[poff : poff + 64, qt, s * 512 : (s + 1) * 512]
                        njt = 4 * (s + 1)
                        yps = ypp.tile([65, 512], f32, tag="y")
                        for grp in range(njt // 2):
                            sp = simp_pool.tile([128, 1024], f32, tag="sim")
                            for jj in range(2):
                                j = grp * 2 + jj
                                r = j - 4 * s
                                # causal: diagonal tile j=4s+r only needs
                                # query columns >= r*128
                                q0 = r * 128 if r > 0 else 0
                                k_ap = qkT[
                                    poff : poff + 64, 2 + qt, j * 128 : (j + 1) * 128
                                ]
                                nc.tensor.matmul(
                                    sp[:, jj * 512 + q0 : (jj + 1) * 512],
                                    k_ap,
                                    q_ap[:, q0:],
                                    start=True,
                                    stop=True,
                                )
                            ex = epool.tile([128, 1024], bf16, tag="exp")
                            nc.scalar.activation(
                                ex[:],
                                sp[:],
                                AF.Exp,
                                bias=zbias[:, 0:1],
                                scale=1.0 / 32.0,
                            )
                            for jj in range(2):
                                j = grp * 2 + jj
                                r = j - 4 * s
                                q0 = r * 128 if r > 0 else 0
                                if 0 <= r < 4:
                                    # zero below-diagonal: keep where f' >= p
                                    # (f' is the offset within [q0:512])
                                    nc.gpsimd.affine_select(
                                        out=ex[:, jj * 512 + q0 : (jj + 1) * 512],
                                        in_=ex[:, jj * 512 + q0 : (jj + 1) * 512],
                                        pattern=[[1, 512 - q0]],
                                        compare_op=mybir.AluOpType.is_ge,
                                        fill=0.0,
                                        base=q0 - r * 128,
                                        channel_multiplier=-1,
                                    )
                                nc.tensor.matmul(
                                    yps[:, q0:],
                                    v1[:, j, l, :],
                                    ex[:, jj * 512 + q0 : (jj + 1) * 512],
                                    start=(j == 0),
                                    stop=(j == njt - 1),
                                    skip_group_check=True,
                                )
                        # normalize: row 64 of yps is the denominator
                        rt = spool.tile([65, 512], f32, tag="rt")
                        nc.vector.reciprocal(rt[64:65, :], yps[64:65, :])
                        if l % 2 == 0:
                            ysl = yta[0:64, qt, s * 512 : (s + 1) * 512]
                        else:
                            ytmp = spool.tile([64, 512], bf16, tag="ytmp")
                            ysl = ytmp[:]
                        # for the very last head the PE is idle anyway and
                        # the engine-local PE-broadcast chain avoids the DMA
                        # hop latency right before the tail projection
                        if use_pbcast and not (s == NS - 1 and li == 3):
                            # partition_broadcast on HW reads the tile's
                            # physical partition 0 - hop the denominator row
                            # down with a tiny SBUF->SBUF DMA first
                            rt0 = spool.tile([1, 512], f32, tag="rt0")
                            nc.sync.dma_start(out=rt0[:], in_=rt[64:65, :])
                            bps = spool.tile([64, 512], f32, tag="bps")
                            nc.gpsimd.partition_broadcast(bps[:], rt0[:])
                            nc.vector.tensor_mul(ysl, yps[0:64, :], bps[:])
                        else:
                            bp = mmp.tile([64, 512], f32, tag="mm")
                            nc.tensor.matmul(
                                bp[:],
                                ones64[64:65, :],
                                rt[64:65, :],
                                start=True,
                                stop=True,
                            )
                            nc.vector.tensor_copy(ysl, yps[0:64, :])
                            nc.vector.tensor_mul(ysl, ysl, bp[:])
                        if l % 2 == 1:
                            nc.sync.dma_start(
                                out=yta[64:128, qt, s * 512 : (s + 1) * 512],
                                in_=ysl,
                            )

                # last super-tile's projection runs at the tail
                emit_proj(NS - 1, (0, 1, 2, 3))

            def emit_proj(s, tts):
                for tt in tts:
                    t = s * 4 + tt
                    ost = opool.tile([128, C], bf16, tag="ost")
                    for n in range(2):
                        pp = mmp.tile([128, 512], f32, tag="mm")
                        for j in range(2):
                            nc.tensor.matmul(
                                pp[:],
                                yta_ref[0][:, j, t * 128 : (t + 1) * 128],
                                wp_ref[0][:, j, n * 512 : (n + 1) * 512],
                                start=(j == 0),
                                stop=(j == 1),
                            )
                        nc.vector.tensor_copy(ost[:, n * 512 : (n + 1) * 512], pp[:])
                    nc.sync.dma_start(
                        out=out_d[t * 128 : (t + 1) * 128, :], in_=ost[:]
                    )

            for _rep in range(repeat):
                emit_once()

    nc.compile()
    return nc


def _get_nc():
    global _cached
    if _cached is None:
        _cached = _build()
    return _cached


def build_in_maps(inputs):
    x = np.asarray(inputs["x"], dtype=np.float32)
    W_attn = np.asarray(inputs["W_attn"], dtype=np.float32)
    b_attn = np.asarray(inputs["b_attn"], dtype=np.float32)
    W_proj = np.asarray(inputs["W_proj"], dtype=np.float32)

    in_maps = []
    for b in range(B):
        xT = np.ascontiguousarray(x[b].T).astype(BF16)
        for g in range(4):
            c0 = g * GC
            wq = W_attn[:, c0 : c0 + GC]
            wk = W_attn[:, C + c0 : C + c0 + GC]
            wqk = np.ascontiguousarray(np.concatenate([wq, wk], axis=1)).astype(BF16)
            wv = np.ascontiguousarray(
                W_attn[:, 2 * C + c0 : 2 * C + c0 + GC]
            ).astype(BF16)
            wp = np.ascontiguousarray(W_proj[c0 : c0 + GC, :]).astype(BF16)
            bqk = np.concatenate(
                [b_attn[c0 : c0 + GC], b_attn[C + c0 : C + c0 + GC]]
            ).astype(np.float32)
            bv = np.ascontiguousarray(
                b_attn[2 * C + c0 : 2 * C + c0 + GC]
            ).astype(np.float32)
            in_maps.append(
                {"xt": xT, "wqk": wqk, "wv": wv, "wp": wp, "bqk": bqk, "bv": bv}
            )
    return in_maps


def kernel(x, W_attn, b_attn, W_proj, b_proj):
    from concourse.bass_utils import run_bass_kernel_spmd

    b_proj = np.asarray(b_proj, dtype=np.float32)
    nc = _get_nc()
    in_maps = build_in_maps(
        {"x": x, "W_attn": W_attn, "b_attn": b_attn, "W_proj": W_proj}
    )
    res = run_bass_kernel_spmd(nc, in_maps, core_ids=list(range(8)))
    out = np.zeros((B, T, C), dtype=np.float32)
    for b in range(B):
        for g in range(4):
            out[b] += res.results[b * 4 + g]["out"].astype(np.float32)
        out[b] += b_proj
    return out

